# revision 1
# baseline (speedup 1.0000x reference)
"""ViT attention block (B=8, N=1024, dim=1024, heads=16, d_k=64) on 8 trn2 NeuronCores.

Sharding: data-parallel over batch (1 batch per core), weights replicated.
No collectives needed; each core computes its batch's full attention output.

Per-core algorithm (all matmuls on TensorE contract over the partition dim):
  - host pre-transposes x[b] -> xT [dim, tokens] so QKV projections can use
    w_qkv (natural layout) as the stationary operand.
  - QT/KT = (w_qkv[:, :2048]).T @ xT  -> [2048, tokens]; head pair 2t,2t+1
    lives in partition-tile t ([128, 1024]), i.e. heads' d_k=64 rows stacked.
  - V = xT.T @ w_qkv[:, 2048:]       -> [tokens, 1024], stored with a
    constant-1 column appended per head (65 cols/head) so the PV matmul
    produces softmax row-sums for free.
  - per head pair: S^T[m,n] = (KT tile).T @ QT (K=64 contraction; the two
    heads run as concurrent row-group matmuls via tile_position).
    exp(scale*S^T) on ScalarE directly out of PSUM -> E^T bf16 in SBUF.
    (max-subtraction is skipped: |scale*S| <~ 2 here, exp is exact-safe and
    softmax is shift-invariant.)
  - PV: out^T[d'+1, n] = V'.T @ E^T accumulated over m tiles; row 64 is the
    softmax denominator. The PSUM tile is staged to SBUF with one copy
    (fast PSUM release); the denominator row is reshaped via DRAM to
    [128, 8] for a full-width reciprocal, broadcast back via a
    partition-broadcast DMA, and fused into the normalize multiply.
  - final = attnT.T @ w_out + b_out, evicted fp32 and DMA'd out.

Schedule: the per-pair loop is software-pipelined to keep TensorE dense
(HAM stays at K=8/8) while ScalarE streams exps:
  slot mt of pair p emits:  QKT filler matmuls for pair p+1,
                            S^T(p, mt) + exp(p, mt),
                            PV(p-1) chunk (h1 in slots 0-3, h2 in 4-7).
"""

import os
import numpy as np
import ml_dtypes

import concourse.bass as bass
from concourse import bacc
import concourse.mybir as mybir
import concourse.tile as tile
from concourse.bass_utils import run_bass_kernel_spmd

P = 128
N_TOK = 1024
DIM = 1024
HEADS = 16
D_K = 64
N_CORES = 8
SCALE = D_K ** -0.5  # 0.125

NP_T = N_TOK // P   # 8 token tiles
DP = DIM // P       # 8 dim tiles
NPAIRS = HEADS // 2  # 8 head pairs
VW = D_K + 1        # 65: V columns per head incl. ones column

# matmul operand dtype: "bf16" | "fp32r" | "fp32"
MM_DTYPE = os.environ.get("KERNEL_MM_DTYPE", "bf16")
_DT = {
    "bf16": mybir.dt.bfloat16,
    "fp32r": mybir.dt.float32r,
    "fp32": mybir.dt.float32,
}[MM_DTYPE]
_NPDT = {"bf16": ml_dtypes.bfloat16, "fp32r": np.float32, "fp32": np.float32}[MM_DTYPE]

F32 = mybir.dt.float32


def build_program():
    nc = bacc.Bacc("TRN2", target_bir_lowering=False, debug=False)

    xT = nc.dram_tensor("xT", [DIM, N_TOK], _DT, kind="ExternalInput").ap()
    wqkv = nc.dram_tensor("w_qkv", [DIM, 3 * DIM], _DT, kind="ExternalInput").ap()
    wout = nc.dram_tensor("w_out", [DIM, DIM], _DT, kind="ExternalInput").ap()
    bout = nc.dram_tensor("b_out", [DIM], F32, kind="ExternalInput").ap()
    out = nc.dram_tensor("out", [N_TOK, DIM], F32, kind="ExternalOutput").ap()
    # denominator bounce buffers (raw row, then reciprocal row)
    rs_dram = nc.dram_tensor("rs_scratch", [HEADS, N_TOK], F32).ap()
    rs2_dram = nc.dram_tensor("rs2_scratch", [HEADS, N_TOK], F32).ap()

    with tile.TileContext(nc) as tc:
        with (
            tc.tile_pool(name="persist", bufs=1) as persist,
            tc.tile_pool(name="qkt", bufs=6) as qktp,
            tc.tile_pool(name="wqk", bufs=24) as wqkp,
        ):
            xT_sb = []
            v_sb = []      # per token-tile: [128, 16*65]
            attnT_sb = []  # per pair: [128, 1024] = two heads' [64, n]
            for j in range(NP_T):
                v_sb.append(persist.tile([P, HEADS * VW], _DT, tag=f"v{j}",
                                         name=f"v{j}"))
            for p in range(NPAIRS):
                attnT_sb.append(persist.tile([P, N_TOK], _DT, tag=f"attnT{p}",
                                             name=f"attnT{p}"))

            def make_qkt_tile(j, pool):
                """Emit QKT M-tile j ([128, tokens] slice of QKV^T) in full."""
                ps = pool.tile([P, N_TOK], F32, tag="pq", name=f"psqk{j}")
                for k in range(DP):
                    w = wqkp.tile([P, P], _DT, tag="wqk", name=f"w{j}_{k}")
                    nc.sync.dma_start(w[:], wqkv[k * P:(k + 1) * P,
                                                 j * P:(j + 1) * P])
                    for nh in range(2):
                        nc.tensor.matmul(
                            ps[:, nh * 512:(nh + 1) * 512],
                            lhsT=w[:],
                            rhs=xT_sb[k][:, nh * 512:(nh + 1) * 512],
                            start=(k == 0), stop=(k == DP - 1),
                        )
                t = qktp.tile([P, N_TOK], _DT, tag="qkt", name=f"qkt{j}")
                nc.vector.tensor_copy(out=t[:], in_=ps[:])
                return t

            # ============ phase 1: V' and pair-0 QT/KT ============
            # V weights live in a scoped pool that frees before ET opens.
            # DMA order matters: interleave xT/wv so V matmuls start early.
            with (
                tc.tile_pool(name="wvp", bufs=1) as wvp,
                tc.tile_pool(name="pq1", bufs=2, space="PSUM") as pq1,
            ):
                wv_sb = []
                for k in range(DP):
                    t = persist.tile([P, N_TOK], _DT, tag=f"xT{k}",
                                     name=f"xT{k}")
                    nc.sync.dma_start(t[:], xT[k * P:(k + 1) * P, :])
                    xT_sb.append(t)
                    w = wvp.tile([P, DIM], _DT, tag=f"wv{k}", name=f"wv{k}")
                    nc.sync.dma_start(w[:], wqkv[k * P:(k + 1) * P, 2 * DIM:])
                    wv_sb.append(w)
                for j in range(NP_T):
                    vt = v_sb[j]
                    nc.vector.memset(
                        vt[:].rearrange("p (h x) -> p h x", x=VW)[:, :, D_K:], 1.0)
                    ps = pq1.tile([P, DIM], F32, tag="pq", name=f"psv{j}")
                    for k in range(DP):
                        for nh in range(2):
                            nc.tensor.matmul(
                                ps[:, nh * 512:(nh + 1) * 512],
                                lhsT=xT_sb[k][:, j * P:(j + 1) * P],
                                rhs=wv_sb[k][:, nh * 512:(nh + 1) * 512],
                                start=(k == 0), stop=(k == DP - 1),
                            )
                    nc.vector.tensor_copy(
                        out=vt[:].rearrange("p (h x) -> p h x", x=VW)[:, :, :D_K],
                        in_=ps[:].rearrange("p (h d) -> p h d", d=D_K),
                    )
                qt_cur = make_qkt_tile(0, pq1)
                kt_cur = make_qkt_tile(DP + 0, pq1)

            # ============ phase 2: pipelined attention ============
            with (
                tc.tile_pool(name="et", bufs=18) as etp,
                tc.tile_pool(name="stg", bufs=3) as stgp,
                tc.tile_pool(name="small", bufs=2) as small,
                tc.tile_pool(name="woutp", bufs=1) as woutp,
                tc.tile_pool(name="ppv", bufs=2, space="PSUM") as ppv,
            ):
                wout_sb = []
                for k in range(DP):
                    w = woutp.tile([P, DIM], _DT, tag=f"wout{k}",
                                   name=f"wout{k}")
                    nc.sync.dma_start(w[:], wout[k * P:(k + 1) * P, :])
                    wout_sb.append(w)
                bias_bc = persist.tile([P, DIM], F32, tag="bias")
                bias_in = bass.AP(tensor=bout.tensor, offset=bout.offset,
                                  ap=[[0, P]] + list(bout.ap))
                nc.sync.dma_start(bias_bc[:], bias_in)
                et_tiles = {}   # (pair, mt) -> ET tile [128, 2048]
                inflight = {}   # accumulation state for pv / filler psums

                def normalize_evict(p, h, stg):
                    """Normalize the staged PV result by the softmax
                    denominator (row 64) and write into attnT_sb[p]."""
                    hg = 2 * p + h
                    # last pair: sync queue is idle by now; keeping its chain
                    # off gpsimd makes the pre-projection DGE drain shallow
                    dma = nc.gpsimd.dma_start if p < NPAIRS - 1 else \
                        nc.sync.dma_start
                    # denominator row -> DRAM -> [128, 8] for wide
                    # reciprocal. These are latency chains, not bandwidth:
                    # keep them off the sync queue (head-of-line blocking of
                    # the streamed weight loads) by using gpsimd SWDGE.
                    dma(rs_dram[hg:hg + 1, :], stg[D_K:VW, :])
                    rsp = small.tile([P, NP_T], F32, tag="rsp", name=f"rsp{hg}")
                    dma(
                        rsp[:], rs_dram[hg].rearrange("(p i) -> p i", p=P))
                    rspr = small.tile([P, NP_T], F32, tag="rspr",
                                      name=f"rspr{hg}")
                    nc.vector.reciprocal(rspr[:], rsp[:])
                    dma(
                        rs2_dram[hg].rearrange("(p i) -> p i", p=P), rspr[:])
                    rs_row = rs2_dram[hg:hg + 1, :]
                    rs_bc = bass.AP(tensor=rs_row.tensor, offset=rs_row.offset,
                                    ap=[[0, D_K], list(rs_row.ap)[-1]])
                    rcp = small.tile([D_K, N_TOK], F32, tag="rcp",
                                     name=f"rcp{hg}")
                    dma(rcp[:], rs_bc)
                    if h == 0:
                        nc.vector.tensor_mul(out=attnT_sb[p][0:D_K, :],
                                             in0=stg[0:D_K, :], in1=rcp[:])
                    else:
                        tmp = small.tile([D_K, N_TOK], _DT, tag="oddtmp",
                                         name=f"oddtmp{hg}")
                        nc.vector.tensor_mul(out=tmp[:],
                                             in0=stg[0:D_K, :], in1=rcp[:])
                        dma(attnT_sb[p][D_K:P, :], tmp[:])

                def pv_chunk(p, slot8):
                    """4 PV matmuls for pair p. Passes of 8 MMs: (h, nh) =
                    slot8//2, each pass covers all m-tiles in 2 slots using a
                    1-bank psum tile; evicted into the pvstage half."""
                    h, nh = slot8 // 4, (slot8 // 2) % 2
                    hg = 2 * p + h
                    half = slot8 % 2  # first or second 4 m-tiles
                    if half == 0:
                        inflight[(p, h, nh)] = ppv.tile(
                            [VW, 512], F32, tag="ppv", name=f"pv{p}_{h}_{nh}")
                    pvt = inflight[(p, h, nh)]
                    for mt in range(4 * half, 4 * half + 4):
                        et = et_tiles[(p, mt)]
                        nc.tensor.matmul(
                            pvt[:],
                            lhsT=v_sb[mt][:, hg * VW:(hg + 1) * VW],
                            rhs=et[:, h * N_TOK + nh * 512:
                                   h * N_TOK + (nh + 1) * 512],
                            start=(mt == 0), stop=(mt == NP_T - 1),
                        )
                    if half == 1:
                        if nh == 0:
                            inflight[("stg", p, h)] = stgp.tile(
                                [VW, N_TOK], F32, tag="stg", name=f"stg{hg}")
                        stg = inflight[("stg", p, h)]
                        nc.vector.tensor_copy(
                            out=stg[:, nh * 512:(nh + 1) * 512], in_=pvt[:])
                        del inflight[(p, h, nh)]
                        if nh == 1:
                            normalize_evict(p, h, stg)
                            del inflight[("stg", p, h)]
                            if h == 1:
                                for mt in range(NP_T):
                                    del et_tiles[(p, mt)]

                def filler_chunk(pnext, mt):
                    """4 QKT matmuls for pair pnext: M-tile qt (mt 0-3) or kt
                    (mt 4-7), k-values 2*(mt%4) and 2*(mt%4)+1, both n-halves.
                    Returns the finished SBUF tile after the 4th chunk."""
                    is_kt = mt >= 4
                    j = (DP + pnext) if is_kt else pnext
                    s = mt % 4
                    key = ("fill", pnext, is_kt)
                    if s == 0:
                        inflight[key] = pq2.tile([P, N_TOK], F32, tag="pq",
                                                 name=f"psf{j}")
                    ps = inflight[key]
                    for k in (2 * s, 2 * s + 1):
                        w = wqkp.tile([P, P], _DT, tag="wqk", name=f"wf{j}_{k}")
                        nc.sync.dma_start(w[:], wqkv[k * P:(k + 1) * P,
                                                     j * P:(j + 1) * P])
                        for nh in range(2):
                            nc.tensor.matmul(
                                ps[:, nh * 512:(nh + 1) * 512],
                                lhsT=w[:],
                                rhs=xT_sb[k][:, nh * 512:(nh + 1) * 512],
                                start=(k == 0), stop=(k == DP - 1),
                            )
                    if s == 3:
                        t = qktp.tile([P, N_TOK], _DT, tag="qkt",
                                      name=f"qkt{j}")
                        nc.vector.tensor_copy(out=t[:], in_=ps[:])
                        del inflight[key]
                        return t
                    return None

                with (
                    tc.tile_pool(name="pst", bufs=1, space="PSUM") as pst,
                    tc.tile_pool(name="pq2", bufs=1, space="PSUM") as pq2,
                ):
                  for p in range(NPAIRS):
                    qt_next = kt_next = None
                    for mt in range(NP_T):
                        # QKT filler for pair p+1
                        if p + 1 < NPAIRS:
                            t = filler_chunk(p + 1, mt)
                            if t is not None:
                                if mt < 4:
                                    qt_next = t
                                else:
                                    kt_next = t
                        # S^T + exp for (p, mt)
                        st = pst.tile([P, 2 * N_TOK], F32, tag="pst",
                                      name=f"st{p}_{mt}")
                        et = etp.tile([P, 2 * N_TOK], _DT, tag="et",
                                      name=f"et{p}_{mt}")
                        for h in range(2):
                            for nh in range(2):
                                nc.tensor.matmul(
                                    st[:, h * N_TOK + nh * 512:
                                       h * N_TOK + (nh + 1) * 512],
                                    lhsT=kt_cur[h * D_K:(h + 1) * D_K,
                                                mt * P:(mt + 1) * P],
                                    rhs=qt_cur[h * D_K:(h + 1) * D_K,
                                               nh * 512:(nh + 1) * 512],
                                    start=True, stop=True,
                                    tile_position=(h * D_K, 0),
                                )
                        nc.scalar.activation(et[:], st[:],
                                             mybir.ActivationFunctionType.Exp,
                                             scale=float(SCALE))
                        et_tiles[(p, mt)] = et
                        # PV chunk for pair p-1
                        if p > 0:
                            pv_chunk(p - 1, mt)
                    qt_cur, kt_cur = qt_next, kt_next

                # ==== drain last pair's PV, overlapped with projection ====
                with (
                    tc.tile_pool(name="ev", bufs=2) as ev,
                    tc.tile_pool(name="pproj", bufs=2, space="PSUM") as pproj,
                ):
                    for slot8 in range(8):
                        pv_chunk(NPAIRS - 1, slot8)
                    # proj: pairs 0..6 accumulate first so the pair-7 chain
                    # latency is hidden; its matmul lands last (stop=True).
                    for j in range(NP_T):
                        ps = pproj.tile([P, DIM], F32, tag="pproj",
                                        name=f"pso{j}")
                        for p in range(NPAIRS):
                            for nh in range(2):
                                nc.tensor.matmul(
                                    ps[:, nh * 512:(nh + 1) * 512],
                                    lhsT=attnT_sb[p][:, j * P:(j + 1) * P],
                                    rhs=wout_sb[p][:, nh * 512:(nh + 1) * 512],
                                    start=(p == 0), stop=(p == NPAIRS - 1),
                                )
                        o = ev.tile([P, DIM], F32, tag="out", name=f"o{j}")
                        nc.vector.tensor_add(out=o[:], in0=ps[:], in1=bias_bc[:])
                        nc.sync.dma_start(out[j * P:(j + 1) * P, :], o[:])

    nc.compile()
    return nc


_NC_CACHE = None


def _get_program():
    global _NC_CACHE
    if _NC_CACHE is None:
        _NC_CACHE = build_program()
    return _NC_CACHE


def make_in_maps(x, w_qkv, w_out, b_out):
    w_qkv_c = np.ascontiguousarray(w_qkv).astype(_NPDT)
    w_out_c = np.ascontiguousarray(w_out).astype(_NPDT)
    b_out_c = np.ascontiguousarray(b_out).astype(np.float32)
    in_maps = []
    for b in range(N_CORES):
        xTb = np.ascontiguousarray(np.asarray(x[b]).T).astype(_NPDT)
        in_maps.append({
            "xT": xTb,
            "w_qkv": w_qkv_c,
            "w_out": w_out_c,
            "b_out": b_out_c,
        })
    return in_maps


def kernel(x, w_qkv, w_out, b_out):
    nc = _get_program()
    in_maps = make_in_maps(x, w_qkv, w_out, b_out)
    res = run_bass_kernel_spmd(nc, in_maps, list(range(N_CORES)))
    outs = [np.asarray(r["out"], dtype=np.float32) for r in res.results]
    return np.stack(outs, axis=0)



# revision 10
# speedup vs baseline: 1.2763x; 1.2763x over previous
"""ViT attention block (B=8, N=1024, dim=1024, heads=16, d_k=64) on 8 trn2 NeuronCores.

Sharding: data-parallel over batch (1 batch per core), weights replicated.
No collectives; each core computes its batch's full attention output.

v2 design (exp-stream centric). Per-core:
  - Q/K projections run in fp8e4 DoubleRow (K=256 per matmul): host ships
    x and w_qkv[:, :2048] as fp8 "dim-pair" tensors [128, 2, *]; w scaled by
    32 (values would be subnormal in e4m3 otherwise), compensated by folding
    1/(32*32) into the exp scale. V projection stays bf16 (fp8 V costs too
    much accuracy).
  - S^T quarters: one matmul = [128 m, 512 n] for one (mt, nh, head); the
    two heads of a pair run as concurrent 64-row-group matmuls. Quarters
    stream into ping-pong PSUM tiles of 3 quarters ([128, 1536], 3 banks x
    2 bufs) so ScalarE's exp (the critical engine, ~1.9us per tile) never
    waits on a PSUM WAR hazard: S^T for tile t+1 fills while exp reads t.
  - exp(scale*S) out of PSUM -> et bf16 in SBUF (max-subtraction skipped:
    |scale*S| <~ 2, exp is exact-safe and softmax shift-invariant).
  - V' = x @ w_v with a constant-1 column per head (65 cols) so PV yields
    softmax row-sums for free; V' matmuls are emitted lazily inside the
    exp phase (PE slack) instead of a serial prologue.
  - PV(p) trails one pair behind the exp stream; [65,512] psum chains over
    8 m-tiles; staged to stg, denominator row reshaped via DRAM for a wide
    reciprocal, broadcast back, fused into the normalize multiply.
  - final = attnT.T @ w_out + b_out in the tail.
"""

import os
import numpy as np
import ml_dtypes

import concourse.bass as bass
from concourse import bacc
import concourse.mybir as mybir
import concourse.tile as tile
from concourse.bass_utils import run_bass_kernel_spmd

P = 128
N_TOK = 1024
DIM = 1024
HEADS = 16
D_K = 64
N_CORES = 8
SCALE = D_K ** -0.5  # 0.125

NP_T = N_TOK // P   # 8 token tiles
DP = DIM // P       # 8 dim tiles
KP = DP // 2        # 4 dim-pair tiles for fp8 DoubleRow
NPAIRS = HEADS // 2  # 8 head pairs
VW = D_K + 1        # 65: V columns per head incl. ones column
W8SCALE = 32.0      # host-side w_qkv fp8 pre-scale (both q and k cols)

NQ = NPAIRS * 32    # 256 S^T quarters ([128, 512] each)
TILE_Q = 3          # quarters per st/et tile
N_ST = (NQ + TILE_Q - 1) // TILE_Q  # 86 tiles (last holds 1 quarter)

BF16 = mybir.dt.bfloat16
F8 = mybir.dt.float8e4
F32 = mybir.dt.float32
DR = mybir.MatmulPerfMode.DoubleRow

# "fp8" (default) = Q/K projection in fp8 DoubleRow; "bf16" = all-bf16
QK_MODE = os.environ.get("KERNEL_QK_MODE", "fp8")


def build_program():
    nc = bacc.Bacc("TRN2", target_bir_lowering=False, debug=False)

    xT = nc.dram_tensor("xT", [DIM, N_TOK], BF16, kind="ExternalInput").ap()
    wv = nc.dram_tensor("w_v", [DIM, DIM], BF16, kind="ExternalInput").ap()
    wout = nc.dram_tensor("w_out", [DIM, DIM], BF16, kind="ExternalInput").ap()
    bout = nc.dram_tensor("b_out", [DIM], F32, kind="ExternalInput").ap()
    if QK_MODE == "fp8":
        x8 = nc.dram_tensor("x8", [KP, P, 2, N_TOK], F8,
                            kind="ExternalInput").ap()
        w8 = nc.dram_tensor("w8", [KP, P, 2, 2 * DIM], F8,
                            kind="ExternalInput").ap()
        exp_scale = float(SCALE) / (W8SCALE * W8SCALE)
    else:
        wqk = nc.dram_tensor("w_qk", [DIM, 2 * DIM], BF16,
                             kind="ExternalInput").ap()
        exp_scale = float(SCALE)
    out = nc.dram_tensor("out", [N_TOK, DIM], F32, kind="ExternalOutput").ap()
    rs_dram = nc.dram_tensor("rs_scratch", [HEADS, N_TOK], F32).ap()
    rs2_dram = nc.dram_tensor("rs2_scratch", [HEADS, N_TOK], F32).ap()
    DEBUG = os.environ.get("KERNEL_DEBUG", "0") == "1"
    if DEBUG:
        dbg_qkt = nc.dram_tensor("dbg_qkt", [2, P, N_TOK], BF16,
                                 kind="ExternalOutput").ap()
        dbg_et = nc.dram_tensor("dbg_et", [3, P, 1536], BF16,
                                kind="ExternalOutput").ap()
        dbg_v = nc.dram_tensor("dbg_v", [P, HEADS * VW], BF16,
                               kind="ExternalOutput").ap()
        dbg_attnT = nc.dram_tensor("dbg_attnT", [P, N_TOK], BF16,
                                   kind="ExternalOutput").ap()
        dbg_stg = nc.dram_tensor("dbg_stg", [2, VW, N_TOK], F32,
                                 kind="ExternalOutput").ap()
        dbg_rcp = nc.dram_tensor("dbg_rcp", [2, D_K, N_TOK], F32,
                                 kind="ExternalOutput").ap()

    with tile.TileContext(nc) as tc:
        with (
            tc.tile_pool(name="persist", bufs=1) as persist,
            tc.tile_pool(name="qkt", bufs=6) as qktp,
            tc.tile_pool(name="etp", bufs=20) as etp,
            tc.tile_pool(name="stg", bufs=3) as stgp,
            tc.tile_pool(name="small", bufs=2) as small,
            tc.tile_pool(name="w8p", bufs=8) as w8p,
        ):
            # ---------------- persistent SBUF ----------------
            x8_sb = []
            if QK_MODE == "fp8":
                for k2 in range(KP):
                    t = persist.tile([P, 2, N_TOK], F8, tag=f"x8_{k2}",
                                     name=f"x8_{k2}")
                    nc.sync.dma_start(t[:], x8[k2])
                    x8_sb.append(t)
            xT_sb = []
            wv_sb = []
            for k in range(DP):
                t = persist.tile([P, N_TOK], BF16, tag=f"xT{k}", name=f"xT{k}")
                nc.sync.dma_start(t[:], xT[k * P:(k + 1) * P, :])
                xT_sb.append(t)
                w = persist.tile([P, DIM], BF16, tag=f"wv{k}", name=f"wv{k}")
                nc.sync.dma_start(w[:], wv[k * P:(k + 1) * P, :])
                wv_sb.append(w)
            v_sb = []
            for j in range(NP_T):
                v_sb.append(persist.tile([P, HEADS * VW], BF16, tag=f"v{j}",
                                         name=f"v{j}"))
            attnT_sb = []
            for p in range(NPAIRS):
                attnT_sb.append(persist.tile([P, N_TOK], BF16, tag=f"attnT{p}",
                                             name=f"attnT{p}"))
            wout_sb = []
            for k in range(DP):
                w = persist.tile([P, DIM], BF16, tag=f"wout{k}",
                                 name=f"wout{k}")
                nc.sync.dma_start(w[:], wout[k * P:(k + 1) * P, :])
                wout_sb.append(w)
            bias_bc = persist.tile([P, DIM], F32, tag="bias")
            bias_in = bass.AP(tensor=bout.tensor, offset=bout.offset,
                              ap=[[0, P]] + list(bout.ap))
            nc.sync.dma_start(bias_bc[:], bias_in)

            st_tiles = {}   # t -> PSUM tile (ping-pong)
            et_tiles = {}   # t -> SBUF bf16 tile
            qkt_done = {}   # ('q'|'k', pair) -> finished [128,1024] bf16 tile
            inflight = {}

            with (
                tc.tile_pool(name="ppv", bufs=1, space="PSUM") as ppv,
                tc.tile_pool(name="pq2", bufs=1, space="PSUM") as pq2,
            ):
                # ---------- QKT M-tile emission (chunk = one nh half) ----
                def qkt_chunk(which, pair, nh):
                    """Emit half of the QT/KT M-tile for `pair`. Returns the
                    finished [128,1024] bf16 tile after the 2nd chunk."""
                    key = (which, pair)
                    colbase = (0 if which == 'q' else DIM) + pair * P
                    if key not in inflight:
                        inflight[key] = qktp.tile([P, N_TOK], BF16, tag="qkt",
                                                  name=f"qkt_{which}{pair}")
                    dest = inflight[key]
                    ps = pq2.tile([P, 512], F32, tag="pq2",
                                  name=f"psqk_{which}{pair}_{nh}")
                    if QK_MODE == "fp8":
                        for k2 in range(KP):
                            w = w8p.tile([P, 2, P], F8, tag="w8",
                                         name=f"w8_{which}{pair}_{nh}_{k2}")
                            nc.sync.dma_start(
                                w[:], w8[k2, :, :, colbase:colbase + P])
                            nc.tensor.matmul(
                                ps[:],
                                lhsT=w[:],
                                rhs=x8_sb[k2][:, :, nh * 512:(nh + 1) * 512],
                                start=(k2 == 0), stop=(k2 == KP - 1),
                                perf_mode=DR,
                            )
                    else:
                        for k in range(DP):
                            w = w8p.tile([P, P], BF16, tag="wqk",
                                         name=f"wqk_{which}{pair}_{nh}_{k}")
                            nc.sync.dma_start(
                                w[:], wqk[k * P:(k + 1) * P,
                                          colbase:colbase + P])
                            nc.tensor.matmul(
                                ps[:],
                                lhsT=w[:],
                                rhs=xT_sb[k][:, nh * 512:(nh + 1) * 512],
                                start=(k == 0), stop=(k == DP - 1),
                            )
                    nc.vector.tensor_copy(
                        out=dest[:, nh * 512:(nh + 1) * 512], in_=ps[:])
                    if nh == 1:
                        del inflight[key]
                        qkt_done[key] = dest
                        return dest
                    return None

                # ---------- V' chunk (j, nh): 8 bf16 matmuls ----------
                def v_chunk(j, nh):
                    if nh == 0:
                        nc.vector.memset(
                            v_sb[j][:].rearrange("p (h x) -> p h x",
                                                 x=VW)[:, :, D_K:], 1.0)
                    ps = pq2.tile([P, 512], F32, tag="pq2", name=f"psv{j}_{nh}")
                    for k in range(DP):
                        nc.tensor.matmul(
                            ps[:],
                            lhsT=xT_sb[k][:, j * P:(j + 1) * P],
                            rhs=wv_sb[k][:, nh * 512:(nh + 1) * 512],
                            start=(k == 0), stop=(k == DP - 1),
                        )
                    hs = nh * (HEADS // 2)
                    nc.vector.tensor_copy(
                        out=v_sb[j][:].rearrange(
                            "p (h x) -> p h x",
                            x=VW)[:, hs:hs + HEADS // 2, :D_K],
                        in_=ps[:].rearrange("p (h d) -> p h d", d=D_K),
                    )

                # ---------- S^T quarter stream ----------
                with tc.tile_pool(name="stp", bufs=2, space="PSUM") as stp:
                    def st_quarter(g, pair, mt, nh, h):
                        t, q = g // TILE_Q, g % TILE_Q
                        if q == 0:
                            width = min(TILE_Q, NQ - t * TILE_Q) * 512
                            st_tiles[t] = stp.tile([P, width], F32, tag="st",
                                                   name=f"st{t}")
                            et_tiles[t] = etp.tile([P, width], BF16, tag="et",
                                                   name=f"et{t}")
                        kt = qkt_done[('k', pair)]
                        qt = qkt_done[('q', pair)]
                        nc.tensor.matmul(
                            st_tiles[t][:, q * 512:(q + 1) * 512],
                            lhsT=kt[h * D_K:(h + 1) * D_K,
                                    mt * P:(mt + 1) * P],
                            rhs=qt[h * D_K:(h + 1) * D_K,
                                   nh * 512:(nh + 1) * 512],
                            start=True, stop=True,
                            tile_position=(h * D_K, 0),
                        )
                        if q == TILE_Q - 1 or g == NQ - 1:
                            nc.scalar.activation(
                                et_tiles[t][:], st_tiles[t][:],
                                mybir.ActivationFunctionType.Exp,
                                scale=exp_scale)
                            del st_tiles[t]
                            if DEBUG and t < 3:
                                nc.sync.dma_start(dbg_et[t], et_tiles[t][:])

                    def et_slice(pair, mt, nh, h):
                        g = 32 * pair + 4 * mt + 2 * nh + h
                        t, q = g // TILE_Q, g % TILE_Q
                        return et_tiles[t][:, q * 512:(q + 1) * 512]

                    # ---------- PV + normalize ----------
                    def normalize_evict(p, h, stg, last):
                        hg = 2 * p + h
                        dma = nc.sync.dma_start if last else \
                            nc.gpsimd.dma_start
                        if DEBUG and p == 0:
                            nc.sync.dma_start(dbg_stg[h], stg[:])
                        dma(rs_dram[hg:hg + 1, :], stg[D_K:VW, :])
                        rsp = small.tile([P, NP_T], F32, tag="rsp",
                                         name=f"rsp{hg}")
                        dma(rsp[:], rs_dram[hg].rearrange("(p i) -> p i", p=P))
                        rspr = small.tile([P, NP_T], F32, tag="rspr",
                                          name=f"rspr{hg}")
                        nc.vector.reciprocal(rspr[:], rsp[:])
                        dma(rs2_dram[hg].rearrange("(p i) -> p i", p=P),
                            rspr[:])
                        rs_row = rs2_dram[hg:hg + 1, :]
                        rs_bc = bass.AP(tensor=rs_row.tensor,
                                        offset=rs_row.offset,
                                        ap=[[0, D_K], list(rs_row.ap)[-1]])
                        rcp = small.tile([D_K, N_TOK], F32, tag="rcp",
                                         name=f"rcp{hg}")
                        dma(rcp[:], rs_bc)
                        if DEBUG and p == 0:
                            nc.sync.dma_start(dbg_rcp[h], rcp[:])
                        if h == 0:
                            nc.vector.tensor_mul(out=attnT_sb[p][0:D_K, :],
                                                 in0=stg[0:D_K, :], in1=rcp[:])
                        else:
                            tmp = small.tile([D_K, N_TOK], BF16, tag="oddtmp",
                                             name=f"oddtmp{hg}")
                            nc.vector.tensor_mul(out=tmp[:],
                                                 in0=stg[0:D_K, :], in1=rcp[:])
                            dma(attnT_sb[p][D_K:P, :], tmp[:])

                    def pv_chunk(p, slot16, last=False):
                        """4 slots per (h, nh) chain: 2 matmuls each."""
                        h, nh = slot16 // 8, (slot16 // 4) % 2
                        hg = 2 * p + h
                        q = slot16 % 4
                        if q == 0:
                            inflight[(p, h, nh)] = ppv.tile(
                                [VW, 512], F32, tag="ppv",
                                name=f"pv{p}_{h}_{nh}")
                        pvt = inflight[(p, h, nh)]
                        for mt in range(2 * q, 2 * q + 2):
                            nc.tensor.matmul(
                                pvt[:],
                                lhsT=v_sb[mt][:, hg * VW:(hg + 1) * VW],
                                rhs=et_slice(p, mt, nh, h),
                                start=(mt == 0), stop=(mt == NP_T - 1),
                            )
                        if q == 3:
                            if nh == 0:
                                inflight[("stg", p, h)] = stgp.tile(
                                    [VW, N_TOK], F32, tag="stg",
                                    name=f"stg{hg}")
                            stg = inflight[("stg", p, h)]
                            nc.vector.tensor_copy(
                                out=stg[:, nh * 512:(nh + 1) * 512],
                                in_=pvt[:])
                            del inflight[(p, h, nh)]
                            if nh == 1:
                                normalize_evict(p, h, stg, last)
                                del inflight[("stg", p, h)]
                                if h == 1:
                                    lo = 32 * p
                                    for t in range(lo // TILE_Q):
                                        et_tiles.pop(t, None)

                    # ================= ramp: pair-0 QT/KT =================
                    for nh in range(2):
                        qkt_chunk('q', 0, nh)
                    for nh in range(2):
                        qkt_chunk('k', 0, nh)

                    if DEBUG:
                        nc.sync.dma_start(dbg_qkt[0], qkt_done[('q', 0)][:])
                        nc.sync.dma_start(dbg_qkt[1], qkt_done[('k', 0)][:])

                    # ================= V' projection =================
                    for j in range(NP_T):
                        for nh in range(2):
                            v_chunk(j, nh)

                    # ================= main pair loop =================
                    for p in range(NPAIRS):
                        for s in range(16):      # slot = (mt, nh)
                            mt, nh = s // 2, s % 2
                            # filler: pair p+1 QT/KT (4 chunks per pair)
                            if p + 1 < NPAIRS and s % 4 == 1:
                                c = s // 4
                                qkt_chunk('q' if c < 2 else 'k', p + 1, c % 2)
                            # S^T quarters (2 concurrent row-group matmuls)
                            for h in range(2):
                                g = 32 * p + 4 * mt + 2 * nh + h
                                st_quarter(g, p, mt, nh, h)
                            # PV for pair p-1 (4 slots per chain)
                            if p > 0:
                                pv_chunk(p - 1, s)

                if DEBUG:
                    nc.sync.dma_start(dbg_v, v_sb[0][:])
                    nc.sync.dma_start(dbg_attnT, attnT_sb[0][:])

                # ============== tail: PV(7) + projection ==============
                # (outside the stp scope so pproj's 4 banks fit)
                with (
                    tc.tile_pool(name="ev", bufs=2) as ev,
                    tc.tile_pool(name="pproj", bufs=2,
                                 space="PSUM") as pproj,
                ):
                    for slot16 in range(16):
                        pv_chunk(NPAIRS - 1, slot16, last=True)
                    for j in range(NP_T):
                        ps = pproj.tile([P, DIM], F32, tag="pproj",
                                        name=f"pso{j}")
                        for p in range(NPAIRS):
                            for nh in range(2):
                                nc.tensor.matmul(
                                    ps[:, nh * 512:(nh + 1) * 512],
                                    lhsT=attnT_sb[p][:, j * P:(j + 1) * P],
                                    rhs=wout_sb[p][:,
                                                   nh * 512:(nh + 1) * 512],
                                    start=(p == 0), stop=(p == NPAIRS - 1),
                                )
                        o = ev.tile([P, DIM], F32, tag="out", name=f"o{j}")
                        nc.vector.tensor_add(out=o[:], in0=ps[:],
                                             in1=bias_bc[:])
                        nc.sync.dma_start(out[j * P:(j + 1) * P, :], o[:])

    nc.compile()
    return nc


_NC_CACHE = None


def _get_program():
    global _NC_CACHE
    if _NC_CACHE is None:
        _NC_CACHE = build_program()
    return _NC_CACHE


def make_in_maps(x, w_qkv, w_out, b_out):
    F8NP = ml_dtypes.float8_e4m3fn
    w_qkv = np.ascontiguousarray(w_qkv).astype(np.float32)
    wv_c = np.ascontiguousarray(w_qkv[:, 2 * DIM:]).astype(ml_dtypes.bfloat16)
    w_out_c = np.ascontiguousarray(w_out).astype(ml_dtypes.bfloat16)
    b_out_c = np.ascontiguousarray(b_out).astype(np.float32)
    common = {
        "w_v": wv_c,
        "w_out": w_out_c,
        "b_out": b_out_c,
    }
    if QK_MODE == "fp8":
        # w8: [KP, 128, 2, 2048], plane i = dim-tile (2*k2 + i)
        wqk8 = (w_qkv[:, :2 * DIM] * W8SCALE).astype(F8NP)
        common["w8"] = np.ascontiguousarray(
            wqk8.reshape(KP, 2, P, 2 * DIM).transpose(0, 2, 1, 3))
    else:
        common["w_qk"] = np.ascontiguousarray(
            w_qkv[:, :2 * DIM]).astype(ml_dtypes.bfloat16)
    in_maps = []
    for b in range(N_CORES):
        xb = np.asarray(x[b], dtype=np.float32)
        xTb = np.ascontiguousarray(xb.T)
        m = dict(common)
        m["xT"] = xTb.astype(ml_dtypes.bfloat16)
        if QK_MODE == "fp8":
            x8b = xTb.astype(F8NP)  # [dim, tok]
            m["x8"] = np.ascontiguousarray(
                x8b.reshape(KP, 2, P, N_TOK).transpose(0, 2, 1, 3))
        in_maps.append(m)
    return in_maps


def kernel(x, w_qkv, w_out, b_out):
    nc = _get_program()
    in_maps = make_in_maps(x, w_qkv, w_out, b_out)
    res = run_bass_kernel_spmd(nc, in_maps, list(range(N_CORES)))
    outs = [np.asarray(r["out"], dtype=np.float32) for r in res.results]
    return np.stack(outs, axis=0)
